# revision 43
# baseline (speedup 1.0000x reference)
"""Bass/Trainium2 kernel for BiDirectionalCrossAttention (8-core SPMD).

Sharding: 8 cores = 4 batches x 2 head-groups (4 heads each).
Per core (batch b, head-group g of 4 heads):
  - Q/K/V projections as fp8e4m3 DoubleRow matmuls (K=256 per matmul:
    channel-chunk pairs), biases added in f32, qt/kt stored bf16
  - V stored fp8 in [token, chan] layout with memset ones-columns
    interleaved per head (softmax denominator rides the attn@V matmul)
  - scoresT[kv, q] per head in bf16 (row-tiled concurrent pairs), exp on
    ScalarE writing fp8e4m3 directly
  - attn@V as fp8 DoubleRow matmuls (K=256: two kv tiles per matmul),
    emitted 2 iterations late so their waits are pre-satisfied (no
    head-of-line blocking on the PE queue)
  - normalization: denominator rows -> reciprocal -> bf16; mid-stream
    units broadcast 1/den via a DRAM bounce (0-stride partition read),
    the final unit via a PE broadcast matmul (latency-critical tail)
  - partial output projection Wout[:, cols_g] @ out_g -> [512, 1024]
Host sums the two partials per batch and adds the folded bias
bout' = bout + Wout @ bv (V-bias commutes through softmax).
"""

import sys
import os

for _p in ("/opt/trn_rl_repo", "/root/.axon_site/_ro/trn_rl_repo"):
    if os.path.isdir(_p) and _p not in sys.path:
        sys.path.append(_p)

import numpy as np
import ml_dtypes

import concourse.bass as bass
import concourse.mybir as mybir
import concourse.tile as tile
from concourse.bass_utils import run_bass_kernel_spmd

BF16 = mybir.dt.bfloat16
F32 = mybir.dt.float32
FP8 = mybir.dt.float8e4
NP_BF16 = ml_dtypes.bfloat16
NP_FP8 = ml_dtypes.float8_e4m3

AF = mybir.ActivationFunctionType
DR = mybir.MatmulPerfMode.DoubleRow


def _split_multi_waits(nc: bass.Bass) -> None:
    """The walrus build here allows only one sync-wait per instruction.
    Tile attaches several; hoist the extras onto same-engine NOPs placed
    immediately before the instruction (same per-engine program order)."""
    uid = 0
    for f in nc.m.functions:
        for bb in f.blocks:
            insts = bb.instructions
            out = []
            changed = False
            for inst in insts:
                si = inst.sync_info
                if si is not None and si.on_wait is not None and len(si.on_wait) > 1:
                    waits = list(si.on_wait)
                    for w in waits[:-1]:
                        nop = mybir.InstNoOp(
                            name=f"splitwait-{uid}",
                            engine=inst.engine,
                            ins=[],
                            outs=[],
                            sync_info=mybir.SyncInfo(on_wait=[w], on_update=[]),
                        )
                        uid += 1
                        out.append(nop)
                    inst.sync_info = mybir.SyncInfo(
                        on_wait=[waits[-1]], on_update=list(si.on_update or [])
                    )
                    changed = True
                out.append(inst)
            if changed:
                bb.instructions = out


def _build_program() -> bass.Bass:
    nc = bass.Bass()

    # host-prepped, partition-contiguous layouts
    qx_d = nc.declare_dram_parameter("qx", [128, 2, 4, 512], FP8, isOutput=False)
    kvx_d = nc.declare_dram_parameter("kvx", [128, 4, 4, 512], FP8, isOutput=False)
    wq_d = nc.declare_dram_parameter("wq", [128, 4, 256], FP8, isOutput=False)
    wk_d = nc.declare_dram_parameter("wk", [128, 4, 256], FP8, isOutput=False)
    wv_d = nc.declare_dram_parameter("wv", [128, 4, 256], FP8, isOutput=False)
    wo_d = nc.declare_dram_parameter("wo", [128, 2, 512], BF16, isOutput=False)
    bq_d = nc.declare_dram_parameter("bq", [128, 2], F32, isOutput=False)
    bk_d = nc.declare_dram_parameter("bk", [128, 2], F32, isOutput=False)
    out_d = nc.declare_dram_parameter("out", [512, 1024], F32, isOutput=True)

    from contextlib import ExitStack

    with tile.TileContext(nc) as tc, ExitStack() as ctx:
        sb = ctx.enter_context(tc.tile_pool(name="sb", bufs=1))
        epool = ctx.enter_context(tc.tile_pool(name="epool", bufs=3))
        small = ctx.enter_context(tc.tile_pool(name="small", bufs=4))
        dpool = ctx.enter_context(tc.tile_pool(name="dram", bufs=2, space="DRAM"))
        # PSUM budget (8 banks): "sc" 2 slots x [128,2,512] (2 banks) = 4,
        # "o" 4 slots x 1 bank = 4 (2 units' accumulators overlap at
        # boundaries; proj psums churn through free slots mid-unit).
        sc_ps = ctx.enter_context(tc.tile_pool(name="scps", bufs=2, space="PSUM"))
        o_ps = ctx.enter_context(tc.tile_pool(name="ops", bufs=4, space="PSUM"))

        # ---------------- SBUF tiles ----------------
        # per-quarter tiles so a consumer's dependency covers only the DMA
        # that actually feeds it (tile-granular dep tracking)
        qx_s = [sb.tile([128, 4, 512], FP8, name=f"qx{h}", tag=f"qx{h}")
                for h in range(2)]
        kvx_s = [sb.tile([128, 4, 512], FP8, name=f"kvx{q}", tag=f"kvx{q}")
                 for q in range(4)]
        wq_s = sb.tile([128, 4, 256], FP8, name="wq", tag="wq")
        wk_s = sb.tile([128, 4, 256], FP8, name="wk", tag="wk")
        wv_s = sb.tile([128, 4, 256], FP8, name="wv", tag="wv")
        wo_s = sb.tile([128, 2, 512], BF16, name="wo", tag="wo")
        bq_s = sb.tile([128, 2], F32, name="bq", tag="bq")
        bk_s = sb.tile([128, 2], F32, name="bk", tag="bk")
        qt_s = [sb.tile([128, 1024], BF16, name=f"qt{m}", tag=f"qt{m}") for m in range(2)]
        kt_s = [sb.tile([128, 2048], BF16, name=f"kt{m}", tag=f"kt{m}") for m in range(2)]
        # v in fp8, [kv-pair, kv-parity, 4 heads x (64 dims + ones col)],
        # padded to 272 so the DoubleRow parity stride is 16B-aligned
        v_s = sb.tile([128, 8, 2, 272], FP8, name="v", tag="v")
        ot_s = [sb.tile([128, 1024], BF16, name=f"ot{m}", tag=f"ot{m}") for m in range(2)]
        # broadcast pattern for the tail norm: bpat[0, 0:64] = 1,
        # bpat[32, 64:128] = 1; rows 1-31 zero. den33 rows 1-31 stay 1.0
        # (recip-safe). Partition writes must be 0/32-aligned (verifier).
        bpat = sb.tile([33, 128], BF16, name="bpat", tag="bpat")
        den33 = sb.tile([33, 512], BF16, name="den33", tag="den33")
        l33 = sb.tile([33, 512], BF16, name="l33", tag="l33")
        recip33b = sb.tile([33, 512], BF16, name="recip33b", tag="recip33b")

        nc.vector.memset(bpat[:], 0.0)
        nc.vector.memset(bpat[0:1, 0:64], 1.0)
        nc.vector.memset(bpat[32:33, 64:128], 1.0)
        nc.vector.memset(den33[:], 1.0)
        for j in range(4):
            nc.vector.memset(v_s[:, :, :, 65 * j + 64], 1.0)
        # prime the ACT table set that holds BOTH exp and ln, so the norm
        # Ln calls never trigger a mid-stream table switch
        nc.scalar.activation(out=l33[0:1, 0:1], in_=den33[0:1, 0:1], func=AF.Ln)

        # ------------- DMAs: consumption order, spread across sequencers ----
        nc.sync.dma_start(out=wk_s[:], in_=wk_d[:])
        nc.sync.dma_start(out=kvx_s[0][:], in_=kvx_d[:, 0])
        nc.sync.dma_start(out=bk_s[:], in_=bk_d[:])
        nc.scalar.dma_start(out=wq_s[:], in_=wq_d[:])
        nc.scalar.dma_start(out=qx_s[0][:], in_=qx_d[:, 0])
        nc.scalar.dma_start(out=bq_s[:], in_=bq_d[:])
        nc.sync.dma_start(out=kvx_s[1][:], in_=kvx_d[:, 1])
        nc.scalar.dma_start(out=qx_s[1][:], in_=qx_d[:, 1])
        nc.sync.dma_start(out=kvx_s[2][:], in_=kvx_d[:, 2])
        nc.scalar.dma_start(out=wv_s[:], in_=wv_d[:])
        nc.sync.dma_start(out=kvx_s[3][:], in_=kvx_d[:, 3])
        nc.scalar.dma_start(out=wo_s[:], in_=wo_d[:])

        # ---------------- building blocks ----------------
        def qproj_group(m, t, bias_engine="vector"):
            ps = o_ps.tile([128, 512], F32, name="o", tag="o", bufs=4)
            for kk in (0, 2):
                nc.tensor.matmul(
                    ps,
                    lhsT=wq_s[:, kk:kk + 2, m * 128:(m + 1) * 128],
                    rhs=qx_s[t][:, kk:kk + 2, :],
                    start=(kk == 0), stop=(kk == 2), perf_mode=DR,
                )
            if bias_engine == "scalar":
                # ScalarE is idle during startup; bias-add there so the
                # first scores don't queue behind DVE
                nc.scalar.activation(
                    out=qt_s[m][:, t * 512:(t + 1) * 512], in_=ps,
                    func=AF.Identity, bias=bq_s[:, m:m + 1],
                )
            else:
                nc.vector.tensor_scalar_add(
                    out=qt_s[m][:, t * 512:(t + 1) * 512], in0=ps,
                    scalar1=bq_s[:, m:m + 1],
                )

        def kproj_group(m, t):
            ps = o_ps.tile([128, 512], F32, name="o", tag="o", bufs=4)
            for kk in (0, 2):
                nc.tensor.matmul(
                    ps,
                    lhsT=wk_s[:, kk:kk + 2, m * 128:(m + 1) * 128],
                    rhs=kvx_s[t][:, kk:kk + 2, :],
                    start=(kk == 0), stop=(kk == 2), perf_mode=DR,
                )
            nc.vector.tensor_scalar_add(
                out=kt_s[m][:, t * 512:(t + 1) * 512], in0=ps,
                scalar1=bk_s[:, m:m + 1],
            )

        def vproj_tile(tt):
            ps = sc_ps.tile([128, 256], F32, name="sc", tag="sc")
            c0 = (tt % 4) * 128
            for kk in (0, 2):
                nc.tensor.matmul(
                    ps,
                    lhsT=kvx_s[tt // 4][:, kk:kk + 2, c0:c0 + 128],
                    rhs=wv_s[:, kk:kk + 2, :],
                    start=(kk == 0), stop=(kk == 2), perf_mode=DR,
                )
            # fp8 cast into the interleaved layout, skipping ones columns
            pair, par = tt // 2, tt % 2
            vsl = v_s[:, pair, par, :]
            v_dst = bass.AP(tensor=vsl.tensor, offset=vsl.offset,
                            ap=[vsl.ap[0], [65, 4], [1, 64]])
            p_src = bass.AP(tensor=ps.tensor, offset=ps.offset,
                            ap=[ps.ap[0], [64, 4], [1, 64]])
            nc.vector.tensor_copy(out=v_dst, in_=p_src)

        o_tiles = {}
        e_pairs = {}
        sd_tiles = {}

        def attn_pair(m, t, j):
            oA, oB = o_tiles[(m, t)]
            e2 = e_pairs.pop((m, t, j))
            jA, jB = 2 * m, 2 * m + 1
            nc.tensor.matmul(
                oA, lhsT=v_s[:, j, :, 65 * jA:65 * jA + 65],
                rhs=e2[:, 0, :, :],
                start=(j == 0), stop=(j == 7), perf_mode=DR,
            )
            nc.tensor.matmul(
                oB, lhsT=v_s[:, j, :, 65 * jB:65 * jB + 65],
                rhs=e2[:, 1, :, :],
                start=(j == 0), stop=(j == 7), perf_mode=DR,
            )

        def _recip_act():
            # 1/den = exp(-ln(den)) on ScalarE — the exp stream has natural
            # gaps at unit boundaries, and this keeps DVE free
            nc.scalar.activation(out=l33, in_=den33, func=AF.Ln)
            nc.scalar.activation(out=recip33b, in_=l33, func=AF.Exp, scale=-1.0)

        def normA(m, t):
            oA, oB = o_tiles[(m, t)]
            nc.vector.tensor_copy(out=den33[0:1, :], in_=oA[64:65, :])
            nc.vector.tensor_copy(out=den33[32:33, :], in_=oB[64:65, :])
            _recip_act()
            sd = dpool.tile([2, 512], BF16, name="sd", tag="sd")
            nc.sync.dma_start(out=sd[0:1, :], in_=recip33b[0:1, :])
            nc.sync.dma_start(out=sd[1:2, :], in_=recip33b[32:33, :])
            sd_tiles[(m, t)] = sd

        def normB(m, t):
            qsl = slice(t * 512, (t + 1) * 512)
            oA, oB = o_tiles.pop((m, t))
            sd = sd_tiles.pop((m, t))
            bcs = small.tile([128, 512], BF16, name="bcs", tag="bcs")
            for row, base in ((0, 0), (1, 64)):
                row_ap = sd[row:row + 1, :]
                bsrc = bass.AP(tensor=row_ap.tensor, offset=row_ap.offset,
                               ap=[[0, 64], [1, 512]])
                nc.sync.dma_start(out=bcs[base:base + 64, :], in_=bsrc)
            nc.vector.tensor_mul(ot_s[m][0:64, qsl], oA[0:64, :], bcs[0:64, :])
            nc.vector.tensor_mul(ot_s[m][64:128, qsl], oB[0:64, :], bcs[64:128, :])

        def norm_tail(m, t):
            qsl = slice(t * 512, (t + 1) * 512)
            oA, oB = o_tiles.pop((m, t))
            nc.scalar.activation(out=den33[0:1, :], in_=oA[64:65, :], func=AF.Copy)
            nc.vector.tensor_copy(out=den33[32:33, :], in_=oB[64:65, :])
            _recip_act()
            # bc borrows the sc pool: it's idle once the exp stream is done
            bc = sc_ps.tile([128, 512], F32, name="sc", tag="sc")
            nc.tensor.matmul(bc, lhsT=bpat[:], rhs=recip33b[:],
                             start=True, stop=True)
            bc_s = small.tile([128, 512], F32, name="bc_s", tag="bc_s")
            nc.vector.tensor_copy(out=bc_s, in_=bc)
            nc.vector.tensor_mul(ot_s[m][0:64, qsl], oA[0:64, :], bc_s[0:64, :])
            nc.vector.tensor_mul(ot_s[m][64:128, qsl], oB[0:64, :], bc_s[64:128, :])

        fo_tiles = {}
        held_ps = {}

        def _outproj_mm(ps, t2, mo, m, start, stop):
            nc.tensor.matmul(
                ps,
                lhsT=wo_s[:, m, mo * 128:(mo + 1) * 128],
                rhs=ot_s[m][:, t2 * 512:(t2 + 1) * 512],
                start=start, stop=stop,
            )

        def _outproj_emit(ps, t2, mo, engine, dma=None):
            if t2 not in fo_tiles:
                fo_tiles[t2] = small.tile([128, 4, 512], F32, name="fo",
                                          tag="fo", bufs=2)
            fo = fo_tiles[t2]
            if engine == "vector":
                nc.vector.tensor_copy(out=fo[:, mo, :], in_=ps)
            else:
                nc.scalar.activation(out=fo[:, mo, :], in_=ps, func=AF.Copy)
            dma_eng = nc.scalar if dma == "scalar" else nc.sync
            dma_eng.dma_start(
                out=out_d[mo * 128:(mo + 1) * 128, t2 * 512:(t2 + 1) * 512],
                in_=fo[:, mo, :],
            )

        def outproj_group(t2, mo, engine="vector", dma=None):
            ps = o_ps.tile([128, 512], F32, name="o", tag="o", bufs=4)
            _outproj_mm(ps, t2, mo, 0, True, False)
            _outproj_mm(ps, t2, mo, 1, False, True)
            _outproj_emit(ps, t2, mo, engine, dma)

        def outproj_first(t2, mo):
            ps = o_ps.tile([128, 512], F32, name="o", tag="o", bufs=4)
            _outproj_mm(ps, t2, mo, 0, True, False)
            held_ps[(t2, mo)] = ps

        def outproj_finish(t2, mo, engine, dma=None):
            ps = held_ps.pop((t2, mo))
            _outproj_mm(ps, t2, mo, 1, False, True)
            _outproj_emit(ps, t2, mo, engine, dma)

        # ---------------- pipelined schedule ----------------
        units = [(0, 0), (1, 0), (0, 1), (1, 1)]
        iters = [(u, i) for u in units for i in range(16)]

        extra = {g: [] for g in range(64)}
        # attn@V pair j of unit u fires at g = 16u + 2j + 3: its exp wait is
        # already satisfied, so it never head-of-line blocks the PE queue.
        tail_attn = []
        for u, (m_, t_) in enumerate(units):
            for j in range(8):
                g = 16 * u + 2 * j + 3
                fn = (lambda m_=m_, t_=t_, j=j: attn_pair(m_, t_, j))
                if g < 64:
                    extra[g].append(fn)
                else:
                    tail_attn.append(fn)
        # vproj(tt) must be emitted before the attn pair that reads it
        # (attn pair j reads tiles 2j, 2j+1 at g=2j+3 — program order is the
        # dependency order)
        for tt in range(14):
            extra[tt].append(lambda tt=tt: vproj_tile(tt))
        extra[15].append(lambda: vproj_tile(14))
        extra[16].append(lambda: vproj_tile(15))
        extra[0].append(lambda: kproj_group(0, 1))
        extra[5].append(lambda: kproj_group(0, 2))
        extra[9].append(lambda: kproj_group(0, 3))
        extra[11].append(lambda: qproj_group(1, 0))
        extra[13].append(lambda: kproj_group(1, 0))
        extra[16].append(lambda: kproj_group(1, 1))
        extra[17].append(lambda: kproj_group(1, 2))
        extra[17].append(lambda: kproj_group(1, 3))
        extra[18].append(lambda: normA(0, 0))
        extra[21].append(lambda: normB(0, 0))
        extra[26].append(lambda: qproj_group(0, 1))
        extra[34].append(lambda: normA(1, 0))
        extra[37].append(lambda: normB(1, 0))
        extra[38].append(lambda: qproj_group(1, 1))
        extra[41].append(lambda: outproj_group(0, 0))
        extra[43].append(lambda: outproj_group(0, 1))
        extra[45].append(lambda: outproj_group(0, 2))
        extra[47].append(lambda: outproj_group(0, 3))
        extra[50].append(lambda: normA(0, 1))
        extra[53].append(lambda: normB(0, 1))
        extra[57].append(lambda: outproj_first(1, 0))
        extra[59].append(lambda: outproj_first(1, 1))

        qproj_group(0, 0, bias_engine="scalar")
        kproj_group(0, 0)

        sc_tiles = {}

        def emit_scores(g):
            (m, t), i = iters[g]
            ksl = slice(i * 128, (i + 1) * 128)
            qsl = slice(t * 512, (t + 1) * 512)
            sc = sc_ps.tile([128, 2, 512], F32, name="sc", tag="sc")
            nc.tensor.matmul(
                sc[:, 0, :], lhsT=kt_s[m][0:64, ksl], rhs=qt_s[m][0:64, qsl],
                start=True, stop=True, tile_position=(0, 0),
            )
            nc.tensor.matmul(
                sc[:, 1, :], lhsT=kt_s[m][64:128, ksl], rhs=qt_s[m][64:128, qsl],
                start=True, stop=True, tile_position=(64, 0),
            )
            sc_tiles[g] = sc

        emit_scores(0)
        e_cur = None
        for g in range(64):
            (m, t), i = iters[g]
            if g + 1 < 64:
                emit_scores(g + 1)
            sc = sc_tiles.pop(g)
            if i % 2 == 0:
                e_cur = epool.tile([128, 2, 2, 512], FP8, name="e", tag="e")
                e_pairs[(m, t, i // 2)] = e_cur
            nc.scalar.activation(out=e_cur[:, :, i % 2, :], in_=sc[:],
                                 func=AF.Exp, scale=0.125)
            if i == 0:
                oA = o_ps.tile([65, 512], F32, name="o", tag="o", bufs=4)
                oB = o_ps.tile([65, 512], F32, name="o", tag="o", bufs=4)
                o_tiles[(m, t)] = (oA, oB)
            for fn in extra.get(g, ()):
                fn()

        for fn in tail_attn:
            fn()
        norm_tail(1, 1)
        outproj_finish(1, 0, "scalar", dma="scalar")
        outproj_finish(1, 1, "vector")
        outproj_group(1, 2, engine="scalar", dma="scalar")
        outproj_group(1, 3, engine="vector")

    _split_multi_waits(nc)
    return nc


_PROGRAM = None


def _get_program() -> bass.Bass:
    global _PROGRAM
    if _PROGRAM is None:
        _PROGRAM = _build_program()
    return _PROGRAM


def _prep_core_inputs(c, q, kv, Wqkv, bqkv, Wout):
    b, g = c // 2, c % 2
    cs = slice(256 * g, 256 * g + 256)
    ks = slice(512 + 256 * g, 512 + 256 * g + 256)
    vs = slice(1024 + 256 * g, 1024 + 256 * g + 256)

    def chunk_w(wT):  # [512, 256] -> [128, 4, 256], row 128k+p -> [p, k]
        return np.ascontiguousarray(wT.reshape(4, 128, -1).transpose(1, 0, 2))

    qxp = q[b].reshape(4, 128, 2, 512).transpose(1, 2, 0, 3)
    kvxp = kv[b].reshape(4, 128, 4, 512).transpose(1, 2, 0, 3)
    return {
        "qx": np.ascontiguousarray(qxp).astype(NP_FP8),
        "kvx": np.ascontiguousarray(kvxp).astype(NP_FP8),
        "wq": chunk_w(Wqkv[cs, :].T).astype(NP_FP8),
        "wk": chunk_w(Wqkv[ks, :].T).astype(NP_FP8),
        "wv": chunk_w(Wqkv[vs, :].T).astype(NP_FP8),
        "wo": np.ascontiguousarray(
            Wout[:, cs].T.reshape(2, 128, 512).transpose(1, 0, 2)).astype(NP_BF16),
        "bq": np.ascontiguousarray(bqkv[cs].reshape(2, 128).T).astype(np.float32),
        "bk": np.ascontiguousarray(bqkv[ks].reshape(2, 128).T).astype(np.float32),
    }


def kernel(q, kv, Wqkv, bqkv, Wout, bout):
    q = np.asarray(q, np.float32)
    kv = np.asarray(kv, np.float32)
    Wqkv = np.asarray(Wqkv, np.float32)
    bqkv = np.asarray(bqkv, np.float32)
    Wout = np.asarray(Wout, np.float32)
    bout = np.asarray(bout, np.float32)

    nc = _get_program()
    in_maps = [_prep_core_inputs(c, q, kv, Wqkv, bqkv, Wout) for c in range(8)]
    res = run_bass_kernel_spmd(nc, in_maps, list(range(8))).results

    # V-bias folds through softmax (rows sum to 1): bout' = bout + Wout @ bv
    bout_adj = bout + Wout @ bqkv[1024:1536]
    out = np.empty((4, 512, 32, 32), np.float32)
    for b in range(4):
        o = res[2 * b]["out"] + res[2 * b + 1]["out"] + bout_adj[:, None]
        out[b] = o.reshape(512, 32, 32)
    return out


# revision 44
# speedup vs baseline: 1.0117x; 1.0117x over previous
"""Bass/Trainium2 kernel for BiDirectionalCrossAttention (8-core SPMD).

Sharding: 8 cores = 4 batches x 2 head-groups (4 heads each).
Per core (batch b, head-group g of 4 heads):
  - Q/K/V projections as fp8e4m3 DoubleRow matmuls (K=256 per matmul:
    channel-chunk pairs), biases added in f32, qt/kt stored bf16
  - V stored fp8 in [token, chan] layout with memset ones-columns
    interleaved per head (softmax denominator rides the attn@V matmul)
  - scoresT[kv, q] per head in bf16 (row-tiled concurrent pairs), exp on
    ScalarE writing fp8e4m3 directly
  - attn@V as fp8 DoubleRow matmuls (K=256: two kv tiles per matmul),
    emitted 2 iterations late so their waits are pre-satisfied (no
    head-of-line blocking on the PE queue)
  - normalization: denominator rows -> reciprocal -> bf16; mid-stream
    units broadcast 1/den via a DRAM bounce (0-stride partition read),
    the final unit via a PE broadcast matmul (latency-critical tail)
  - partial output projection Wout[:, cols_g] @ out_g -> [512, 1024]
Host sums the two partials per batch and adds the folded bias
bout' = bout + Wout @ bv (V-bias commutes through softmax).
"""

import sys
import os

for _p in ("/opt/trn_rl_repo", "/root/.axon_site/_ro/trn_rl_repo"):
    if os.path.isdir(_p) and _p not in sys.path:
        sys.path.append(_p)

import numpy as np
import ml_dtypes

import concourse.bass as bass
import concourse.mybir as mybir
import concourse.tile as tile
from concourse.bass_utils import run_bass_kernel_spmd

BF16 = mybir.dt.bfloat16
F32 = mybir.dt.float32
FP8 = mybir.dt.float8e4
NP_BF16 = ml_dtypes.bfloat16
NP_FP8 = ml_dtypes.float8_e4m3

AF = mybir.ActivationFunctionType
DR = mybir.MatmulPerfMode.DoubleRow


def _split_multi_waits(nc: bass.Bass) -> None:
    """The walrus build here allows only one sync-wait per instruction.
    Tile attaches several; hoist the extras onto same-engine NOPs placed
    immediately before the instruction (same per-engine program order)."""
    uid = 0
    for f in nc.m.functions:
        for bb in f.blocks:
            insts = bb.instructions
            out = []
            changed = False
            for inst in insts:
                si = inst.sync_info
                if si is not None and si.on_wait is not None and len(si.on_wait) > 1:
                    waits = list(si.on_wait)
                    for w in waits[:-1]:
                        nop = mybir.InstNoOp(
                            name=f"splitwait-{uid}",
                            engine=inst.engine,
                            ins=[],
                            outs=[],
                            sync_info=mybir.SyncInfo(on_wait=[w], on_update=[]),
                        )
                        uid += 1
                        out.append(nop)
                    inst.sync_info = mybir.SyncInfo(
                        on_wait=[waits[-1]], on_update=list(si.on_update or [])
                    )
                    changed = True
                out.append(inst)
            if changed:
                bb.instructions = out


def _build_program() -> bass.Bass:
    nc = bass.Bass()

    # host-prepped, partition-contiguous layouts
    qx_d = nc.declare_dram_parameter("qx", [128, 2, 4, 512], FP8, isOutput=False)
    kvx_d = nc.declare_dram_parameter("kvx", [128, 4, 4, 512], FP8, isOutput=False)
    wq_d = nc.declare_dram_parameter("wq", [128, 4, 256], FP8, isOutput=False)
    wk_d = nc.declare_dram_parameter("wk", [128, 4, 256], FP8, isOutput=False)
    wv_d = nc.declare_dram_parameter("wv", [128, 4, 256], FP8, isOutput=False)
    wo_d = nc.declare_dram_parameter("wo", [128, 2, 512], BF16, isOutput=False)
    bq_d = nc.declare_dram_parameter("bq", [128, 2], F32, isOutput=False)
    bk_d = nc.declare_dram_parameter("bk", [128, 2], F32, isOutput=False)
    out_d = nc.declare_dram_parameter("out", [512, 1024], F32, isOutput=True)

    from contextlib import ExitStack

    with tile.TileContext(nc) as tc, ExitStack() as ctx:
        sb = ctx.enter_context(tc.tile_pool(name="sb", bufs=1))
        epool = ctx.enter_context(tc.tile_pool(name="epool", bufs=3))
        small = ctx.enter_context(tc.tile_pool(name="small", bufs=4))
        dpool = ctx.enter_context(tc.tile_pool(name="dram", bufs=2, space="DRAM"))
        # PSUM budget (8 banks): "sc" 2 slots x [128,2,512] (2 banks) = 4,
        # "o" 4 slots x 1 bank = 4 (2 units' accumulators overlap at
        # boundaries; proj psums churn through free slots mid-unit).
        sc_ps = ctx.enter_context(tc.tile_pool(name="scps", bufs=2, space="PSUM"))
        o_ps = ctx.enter_context(tc.tile_pool(name="ops", bufs=4, space="PSUM"))

        # ---------------- SBUF tiles ----------------
        # per-quarter tiles so a consumer's dependency covers only the DMA
        # that actually feeds it (tile-granular dep tracking)
        qx_s = [sb.tile([128, 4, 512], FP8, name=f"qx{h}", tag=f"qx{h}")
                for h in range(2)]
        kvx_s = [sb.tile([128, 4, 512], FP8, name=f"kvx{q}", tag=f"kvx{q}")
                 for q in range(4)]
        wq_s = sb.tile([128, 4, 256], FP8, name="wq", tag="wq")
        wk_s = sb.tile([128, 4, 256], FP8, name="wk", tag="wk")
        wv_s = sb.tile([128, 4, 256], FP8, name="wv", tag="wv")
        wo_s = sb.tile([128, 2, 512], BF16, name="wo", tag="wo")
        bq_s = sb.tile([128, 2], F32, name="bq", tag="bq")
        bk_s = sb.tile([128, 2], F32, name="bk", tag="bk")
        qt_s = [sb.tile([128, 1024], BF16, name=f"qt{m}", tag=f"qt{m}") for m in range(2)]
        kt_s = [sb.tile([128, 2048], BF16, name=f"kt{m}", tag=f"kt{m}") for m in range(2)]
        # v in fp8, [kv-pair, kv-parity, 4 heads x (64 dims + ones col)],
        # padded to 272 so the DoubleRow parity stride is 16B-aligned
        v_s = sb.tile([128, 8, 2, 272], FP8, name="v", tag="v")
        ot_s = [sb.tile([128, 1024], BF16, name=f"ot{m}", tag=f"ot{m}") for m in range(2)]
        # broadcast pattern for the tail norm: bpat[0, 0:64] = 1,
        # bpat[32, 64:128] = 1; rows 1-31 zero. den33 rows 1-31 stay 1.0
        # (recip-safe). Partition writes must be 0/32-aligned (verifier).
        bpat = sb.tile([33, 128], BF16, name="bpat", tag="bpat")
        den33 = sb.tile([33, 512], BF16, name="den33", tag="den33")
        l33 = sb.tile([33, 512], BF16, name="l33", tag="l33")
        recip33b = sb.tile([33, 512], BF16, name="recip33b", tag="recip33b")

        nc.vector.memset(bpat[:], 0.0)
        nc.vector.memset(bpat[0:1, 0:64], 1.0)
        nc.vector.memset(bpat[32:33, 64:128], 1.0)
        nc.vector.memset(den33[:], 1.0)
        for j in range(4):
            nc.vector.memset(v_s[:, :, :, 65 * j + 64], 1.0)
        # prime the ACT table set that holds BOTH exp and ln, so the norm
        # Ln calls never trigger a mid-stream table switch
        nc.scalar.activation(out=l33[0:1, 0:1], in_=den33[0:1, 0:1], func=AF.Ln)

        # ------------- DMAs: consumption order, spread across sequencers ----
        nc.sync.dma_start(out=wk_s[:], in_=wk_d[:])
        nc.sync.dma_start(out=kvx_s[0][:], in_=kvx_d[:, 0])
        nc.sync.dma_start(out=bk_s[:], in_=bk_d[:])
        nc.scalar.dma_start(out=wq_s[:], in_=wq_d[:])
        nc.scalar.dma_start(out=qx_s[0][:], in_=qx_d[:, 0])
        nc.scalar.dma_start(out=bq_s[:], in_=bq_d[:])
        nc.sync.dma_start(out=kvx_s[1][:], in_=kvx_d[:, 1])
        nc.scalar.dma_start(out=qx_s[1][:], in_=qx_d[:, 1])
        nc.sync.dma_start(out=kvx_s[2][:], in_=kvx_d[:, 2])
        nc.scalar.dma_start(out=wv_s[:], in_=wv_d[:])
        nc.sync.dma_start(out=kvx_s[3][:], in_=kvx_d[:, 3])
        nc.scalar.dma_start(out=wo_s[:], in_=wo_d[:])

        # ---------------- building blocks ----------------
        def qproj_group(m, t, bias_engine="vector"):
            ps = o_ps.tile([128, 512], F32, name="o", tag="o", bufs=4)
            for kk in (0, 2):
                nc.tensor.matmul(
                    ps,
                    lhsT=wq_s[:, kk:kk + 2, m * 128:(m + 1) * 128],
                    rhs=qx_s[t][:, kk:kk + 2, :],
                    start=(kk == 0), stop=(kk == 2), perf_mode=DR,
                )
            if bias_engine == "scalar":
                # ScalarE is idle during startup; bias-add there so the
                # first scores don't queue behind DVE
                nc.scalar.activation(
                    out=qt_s[m][:, t * 512:(t + 1) * 512], in_=ps,
                    func=AF.Identity, bias=bq_s[:, m:m + 1],
                )
            else:
                nc.vector.tensor_scalar_add(
                    out=qt_s[m][:, t * 512:(t + 1) * 512], in0=ps,
                    scalar1=bq_s[:, m:m + 1],
                )

        def kproj_group(m, t):
            ps = o_ps.tile([128, 512], F32, name="o", tag="o", bufs=4)
            for kk in (0, 2):
                nc.tensor.matmul(
                    ps,
                    lhsT=wk_s[:, kk:kk + 2, m * 128:(m + 1) * 128],
                    rhs=kvx_s[t][:, kk:kk + 2, :],
                    start=(kk == 0), stop=(kk == 2), perf_mode=DR,
                )
            nc.vector.tensor_scalar_add(
                out=kt_s[m][:, t * 512:(t + 1) * 512], in0=ps,
                scalar1=bk_s[:, m:m + 1],
            )

        def vproj_tile(tt):
            ps = sc_ps.tile([128, 256], F32, name="sc", tag="sc")
            c0 = (tt % 4) * 128
            for kk in (0, 2):
                nc.tensor.matmul(
                    ps,
                    lhsT=kvx_s[tt // 4][:, kk:kk + 2, c0:c0 + 128],
                    rhs=wv_s[:, kk:kk + 2, :],
                    start=(kk == 0), stop=(kk == 2), perf_mode=DR,
                )
            # fp8 cast into the interleaved layout, skipping ones columns
            pair, par = tt // 2, tt % 2
            vsl = v_s[:, pair, par, :]
            v_dst = bass.AP(tensor=vsl.tensor, offset=vsl.offset,
                            ap=[vsl.ap[0], [65, 4], [1, 64]])
            p_src = bass.AP(tensor=ps.tensor, offset=ps.offset,
                            ap=[ps.ap[0], [64, 4], [1, 64]])
            nc.vector.tensor_copy(out=v_dst, in_=p_src)

        o_tiles = {}
        e_pairs = {}
        sd_tiles = {}

        def attn_pair(m, t, j):
            oA, oB = o_tiles[(m, t)]
            e2 = e_pairs.pop((m, t, j))
            jA, jB = 2 * m, 2 * m + 1
            nc.tensor.matmul(
                oA, lhsT=v_s[:, j, :, 65 * jA:65 * jA + 65],
                rhs=e2[:, 0, :, :],
                start=(j == 0), stop=(j == 7), perf_mode=DR,
            )
            nc.tensor.matmul(
                oB, lhsT=v_s[:, j, :, 65 * jB:65 * jB + 65],
                rhs=e2[:, 1, :, :],
                start=(j == 0), stop=(j == 7), perf_mode=DR,
            )

        def _recip_act():
            # 1/den = exp(-ln(den)) on ScalarE — the exp stream has natural
            # gaps at unit boundaries, and this keeps DVE free
            nc.scalar.activation(out=l33, in_=den33, func=AF.Ln)
            nc.scalar.activation(out=recip33b, in_=l33, func=AF.Exp, scale=-1.0)

        def normA(m, t):
            oA, oB = o_tiles[(m, t)]
            nc.vector.tensor_copy(out=den33[0:1, :], in_=oA[64:65, :])
            nc.vector.tensor_copy(out=den33[32:33, :], in_=oB[64:65, :])
            _recip_act()
            sd = dpool.tile([2, 512], BF16, name="sd", tag="sd")
            nc.sync.dma_start(out=sd[0:1, :], in_=recip33b[0:1, :])
            nc.sync.dma_start(out=sd[1:2, :], in_=recip33b[32:33, :])
            sd_tiles[(m, t)] = sd

        def normB(m, t):
            qsl = slice(t * 512, (t + 1) * 512)
            oA, oB = o_tiles.pop((m, t))
            sd = sd_tiles.pop((m, t))
            bcs = small.tile([128, 512], BF16, name="bcs", tag="bcs")
            for row, base in ((0, 0), (1, 64)):
                row_ap = sd[row:row + 1, :]
                bsrc = bass.AP(tensor=row_ap.tensor, offset=row_ap.offset,
                               ap=[[0, 64], [1, 512]])
                nc.sync.dma_start(out=bcs[base:base + 64, :], in_=bsrc)
            nc.vector.tensor_mul(ot_s[m][0:64, qsl], oA[0:64, :], bcs[0:64, :])
            nc.vector.tensor_mul(ot_s[m][64:128, qsl], oB[0:64, :], bcs[64:128, :])

        def norm_tail(m, t):
            qsl = slice(t * 512, (t + 1) * 512)
            oA, oB = o_tiles.pop((m, t))
            nc.scalar.activation(out=den33[0:1, :], in_=oA[64:65, :], func=AF.Copy)
            nc.vector.tensor_copy(out=den33[32:33, :], in_=oB[64:65, :])
            _recip_act()
            # bc borrows the sc pool: it's idle once the exp stream is done
            bc = sc_ps.tile([128, 512], F32, name="sc", tag="sc")
            nc.tensor.matmul(bc, lhsT=bpat[:], rhs=recip33b[:],
                             start=True, stop=True)
            bc_s = small.tile([128, 512], F32, name="bc_s", tag="bc_s")
            nc.vector.tensor_copy(out=bc_s, in_=bc)
            nc.vector.tensor_mul(ot_s[m][0:64, qsl], oA[0:64, :], bc_s[0:64, :])
            nc.vector.tensor_mul(ot_s[m][64:128, qsl], oB[0:64, :], bc_s[64:128, :])

        fo_tiles = {}
        held_ps = {}

        def _outproj_mm(ps, t2, mo, m, start, stop):
            nc.tensor.matmul(
                ps,
                lhsT=wo_s[:, m, mo * 128:(mo + 1) * 128],
                rhs=ot_s[m][:, t2 * 512:(t2 + 1) * 512],
                start=start, stop=stop,
            )

        def _outproj_emit(ps, t2, mo, engine, dma=None):
            if t2 not in fo_tiles:
                fo_tiles[t2] = small.tile([128, 4, 512], F32, name="fo",
                                          tag="fo", bufs=2)
            fo = fo_tiles[t2]
            if engine == "vector":
                nc.vector.tensor_copy(out=fo[:, mo, :], in_=ps)
            else:
                nc.scalar.activation(out=fo[:, mo, :], in_=ps, func=AF.Copy)
            dma_eng = nc.scalar if dma == "scalar" else nc.sync
            dma_eng.dma_start(
                out=out_d[mo * 128:(mo + 1) * 128, t2 * 512:(t2 + 1) * 512],
                in_=fo[:, mo, :],
            )

        def outproj_group(t2, mo, engine="vector", dma=None):
            ps = o_ps.tile([128, 512], F32, name="o", tag="o", bufs=4)
            _outproj_mm(ps, t2, mo, 0, True, False)
            _outproj_mm(ps, t2, mo, 1, False, True)
            _outproj_emit(ps, t2, mo, engine, dma)

        def outproj_first(t2, mo):
            ps = o_ps.tile([128, 512], F32, name="o", tag="o", bufs=4)
            _outproj_mm(ps, t2, mo, 0, True, False)
            held_ps[(t2, mo)] = ps

        def outproj_finish(t2, mo, engine, dma=None):
            ps = held_ps.pop((t2, mo))
            _outproj_mm(ps, t2, mo, 1, False, True)
            _outproj_emit(ps, t2, mo, engine, dma)

        # ---------------- pipelined schedule ----------------
        units = [(0, 0), (1, 0), (0, 1), (1, 1)]
        iters = [(u, i) for u in units for i in range(16)]

        extra = {g: [] for g in range(64)}
        # attn@V pair j of unit u fires at g = 16u + 2j + 3: its exp wait is
        # already satisfied, so it never head-of-line blocks the PE queue.
        tail_attn = []
        for u, (m_, t_) in enumerate(units):
            for j in range(8):
                g = 16 * u + 2 * j + 3
                fn = (lambda m_=m_, t_=t_, j=j: attn_pair(m_, t_, j))
                if g < 64:
                    extra[g].append(fn)
                else:
                    tail_attn.append(fn)
        # vproj(tt) must be emitted before the attn pair that reads it
        # (attn pair j reads tiles 2j, 2j+1 at g=2j+3 — program order is the
        # dependency order)
        for tt in range(14):
            extra[tt].append(lambda tt=tt: vproj_tile(tt))
        extra[15].append(lambda: vproj_tile(14))
        extra[16].append(lambda: vproj_tile(15))
        extra[1].append(lambda: kproj_group(0, 1))
        extra[5].append(lambda: kproj_group(0, 2))
        extra[9].append(lambda: kproj_group(0, 3))
        extra[11].append(lambda: qproj_group(1, 0))
        extra[13].append(lambda: kproj_group(1, 0))
        extra[16].append(lambda: kproj_group(1, 1))
        extra[17].append(lambda: kproj_group(1, 2))
        extra[17].append(lambda: kproj_group(1, 3))
        extra[18].append(lambda: normA(0, 0))
        extra[21].append(lambda: normB(0, 0))
        extra[26].append(lambda: qproj_group(0, 1))
        extra[34].append(lambda: normA(1, 0))
        extra[37].append(lambda: normB(1, 0))
        extra[38].append(lambda: qproj_group(1, 1))
        extra[41].append(lambda: outproj_group(0, 0))
        extra[43].append(lambda: outproj_group(0, 1))
        extra[45].append(lambda: outproj_group(0, 2))
        extra[47].append(lambda: outproj_group(0, 3))
        extra[50].append(lambda: normA(0, 1))
        extra[53].append(lambda: normB(0, 1))
        extra[57].append(lambda: outproj_first(1, 0))
        extra[59].append(lambda: outproj_first(1, 1))

        qproj_group(0, 0, bias_engine="scalar")
        kproj_group(0, 0)

        sc_tiles = {}

        def emit_scores(g):
            (m, t), i = iters[g]
            ksl = slice(i * 128, (i + 1) * 128)
            qsl = slice(t * 512, (t + 1) * 512)
            sc = sc_ps.tile([128, 2, 512], F32, name="sc", tag="sc")
            nc.tensor.matmul(
                sc[:, 0, :], lhsT=kt_s[m][0:64, ksl], rhs=qt_s[m][0:64, qsl],
                start=True, stop=True, tile_position=(0, 0),
            )
            nc.tensor.matmul(
                sc[:, 1, :], lhsT=kt_s[m][64:128, ksl], rhs=qt_s[m][64:128, qsl],
                start=True, stop=True, tile_position=(64, 0),
            )
            sc_tiles[g] = sc

        emit_scores(0)
        e_cur = None
        for g in range(64):
            (m, t), i = iters[g]
            if g + 1 < 64:
                emit_scores(g + 1)
            sc = sc_tiles.pop(g)
            if i % 2 == 0:
                e_cur = epool.tile([128, 2, 2, 512], FP8, name="e", tag="e")
                e_pairs[(m, t, i // 2)] = e_cur
            nc.scalar.activation(out=e_cur[:, :, i % 2, :], in_=sc[:],
                                 func=AF.Exp, scale=0.125)
            if i == 0:
                oA = o_ps.tile([65, 512], F32, name="o", tag="o", bufs=4)
                oB = o_ps.tile([65, 512], F32, name="o", tag="o", bufs=4)
                o_tiles[(m, t)] = (oA, oB)
            for fn in extra.get(g, ()):
                fn()

        for fn in tail_attn:
            fn()
        norm_tail(1, 1)
        outproj_finish(1, 0, "scalar", dma="scalar")
        outproj_finish(1, 1, "vector")
        outproj_group(1, 2, engine="scalar", dma="scalar")
        outproj_group(1, 3, engine="vector")

    _split_multi_waits(nc)
    return nc


_PROGRAM = None


def _get_program() -> bass.Bass:
    global _PROGRAM
    if _PROGRAM is None:
        _PROGRAM = _build_program()
    return _PROGRAM


def _prep_core_inputs(c, q, kv, Wqkv, bqkv, Wout):
    b, g = c // 2, c % 2
    cs = slice(256 * g, 256 * g + 256)
    ks = slice(512 + 256 * g, 512 + 256 * g + 256)
    vs = slice(1024 + 256 * g, 1024 + 256 * g + 256)

    def chunk_w(wT):  # [512, 256] -> [128, 4, 256], row 128k+p -> [p, k]
        return np.ascontiguousarray(wT.reshape(4, 128, -1).transpose(1, 0, 2))

    qxp = q[b].reshape(4, 128, 2, 512).transpose(1, 2, 0, 3)
    kvxp = kv[b].reshape(4, 128, 4, 512).transpose(1, 2, 0, 3)
    return {
        "qx": np.ascontiguousarray(qxp).astype(NP_FP8),
        "kvx": np.ascontiguousarray(kvxp).astype(NP_FP8),
        "wq": chunk_w(Wqkv[cs, :].T).astype(NP_FP8),
        "wk": chunk_w(Wqkv[ks, :].T).astype(NP_FP8),
        "wv": chunk_w(Wqkv[vs, :].T).astype(NP_FP8),
        "wo": np.ascontiguousarray(
            Wout[:, cs].T.reshape(2, 128, 512).transpose(1, 0, 2)).astype(NP_BF16),
        "bq": np.ascontiguousarray(bqkv[cs].reshape(2, 128).T).astype(np.float32),
        "bk": np.ascontiguousarray(bqkv[ks].reshape(2, 128).T).astype(np.float32),
    }


def kernel(q, kv, Wqkv, bqkv, Wout, bout):
    q = np.asarray(q, np.float32)
    kv = np.asarray(kv, np.float32)
    Wqkv = np.asarray(Wqkv, np.float32)
    bqkv = np.asarray(bqkv, np.float32)
    Wout = np.asarray(Wout, np.float32)
    bout = np.asarray(bout, np.float32)

    nc = _get_program()
    in_maps = [_prep_core_inputs(c, q, kv, Wqkv, bqkv, Wout) for c in range(8)]
    res = run_bass_kernel_spmd(nc, in_maps, list(range(8))).results

    # V-bias folds through softmax (rows sum to 1): bout' = bout + Wout @ bv
    bout_adj = bout + Wout @ bqkv[1024:1536]
    out = np.empty((4, 512, 32, 32), np.float32)
    for b in range(4):
        o = res[2 * b]["out"] + res[2 * b + 1]["out"] + bout_adj[:, None]
        out[b] = o.reshape(512, 32, 32)
    return out


# revision 48
# speedup vs baseline: 1.0223x; 1.0104x over previous
"""Bass/Trainium2 kernel for BiDirectionalCrossAttention (8-core SPMD).

Sharding: 8 cores = 4 batches x 2 head-groups (4 heads each).
Per core (batch b, head-group g of 4 heads):
  - Q/K/V projections as fp8e4m3 DoubleRow matmuls (K=256 per matmul:
    channel-chunk pairs), biases added in f32, qt/kt stored bf16
  - V stored fp8 in [token, chan] layout with memset ones-columns
    interleaved per head (softmax denominator rides the attn@V matmul)
  - scoresT[kv, q] per head in bf16 (row-tiled concurrent pairs), exp on
    ScalarE writing fp8e4m3 directly
  - attn@V as fp8 DoubleRow matmuls (K=256: two kv tiles per matmul),
    emitted 2 iterations late so their waits are pre-satisfied (no
    head-of-line blocking on the PE queue)
  - normalization: denominator rows -> reciprocal -> bf16; mid-stream
    units broadcast 1/den via a DRAM bounce (0-stride partition read),
    the final unit via a PE broadcast matmul (latency-critical tail)
  - partial output projection Wout[:, cols_g] @ out_g -> [512, 1024]
Host sums the two partials per batch and adds the folded bias
bout' = bout + Wout @ bv (V-bias commutes through softmax).
"""

import sys
import os

for _p in ("/opt/trn_rl_repo", "/root/.axon_site/_ro/trn_rl_repo"):
    if os.path.isdir(_p) and _p not in sys.path:
        sys.path.append(_p)

import numpy as np
import ml_dtypes

import concourse.bass as bass
import concourse.mybir as mybir
import concourse.tile as tile
from concourse.bass_utils import run_bass_kernel_spmd

BF16 = mybir.dt.bfloat16
F32 = mybir.dt.float32
FP8 = mybir.dt.float8e4
NP_BF16 = ml_dtypes.bfloat16
NP_FP8 = ml_dtypes.float8_e4m3

AF = mybir.ActivationFunctionType
DR = mybir.MatmulPerfMode.DoubleRow


def _split_multi_waits(nc: bass.Bass) -> None:
    """The walrus build here allows only one sync-wait per instruction.
    Tile attaches several; hoist the extras onto same-engine NOPs placed
    immediately before the instruction (same per-engine program order)."""
    uid = 0
    for f in nc.m.functions:
        for bb in f.blocks:
            insts = bb.instructions
            out = []
            changed = False
            for inst in insts:
                si = inst.sync_info
                if si is not None and si.on_wait is not None and len(si.on_wait) > 1:
                    waits = list(si.on_wait)
                    for w in waits[:-1]:
                        nop = mybir.InstNoOp(
                            name=f"splitwait-{uid}",
                            engine=inst.engine,
                            ins=[],
                            outs=[],
                            sync_info=mybir.SyncInfo(on_wait=[w], on_update=[]),
                        )
                        uid += 1
                        out.append(nop)
                    inst.sync_info = mybir.SyncInfo(
                        on_wait=[waits[-1]], on_update=list(si.on_update or [])
                    )
                    changed = True
                out.append(inst)
            if changed:
                bb.instructions = out


def _build_program() -> bass.Bass:
    nc = bass.Bass()

    # host-prepped, partition-contiguous layouts
    qx_d = nc.declare_dram_parameter("qx", [128, 2, 4, 512], FP8, isOutput=False)
    kvx_d = nc.declare_dram_parameter("kvx", [128, 4, 4, 512], FP8, isOutput=False)
    wq_d = nc.declare_dram_parameter("wq", [128, 4, 256], FP8, isOutput=False)
    wk_d = nc.declare_dram_parameter("wk", [128, 4, 256], FP8, isOutput=False)
    wv_d = nc.declare_dram_parameter("wv", [128, 4, 256], FP8, isOutput=False)
    wo_d = nc.declare_dram_parameter("wo", [128, 2, 512], BF16, isOutput=False)
    bq_d = nc.declare_dram_parameter("bq", [128, 2], F32, isOutput=False)
    bk_d = nc.declare_dram_parameter("bk", [128, 2], F32, isOutput=False)
    out_d = nc.declare_dram_parameter("out", [512, 1024], F32, isOutput=True)

    from contextlib import ExitStack

    with tile.TileContext(nc) as tc, ExitStack() as ctx:
        sb = ctx.enter_context(tc.tile_pool(name="sb", bufs=1))
        epool = ctx.enter_context(tc.tile_pool(name="epool", bufs=3))
        small = ctx.enter_context(tc.tile_pool(name="small", bufs=4))
        dpool = ctx.enter_context(tc.tile_pool(name="dram", bufs=2, space="DRAM"))
        # PSUM budget (8 banks): "sc" 2 slots x [128,2,512] (2 banks) = 4,
        # "o" 4 slots x 1 bank = 4 (2 units' accumulators overlap at
        # boundaries; proj psums churn through free slots mid-unit).
        sc_ps = ctx.enter_context(tc.tile_pool(name="scps", bufs=2, space="PSUM"))
        o_ps = ctx.enter_context(tc.tile_pool(name="ops", bufs=4, space="PSUM"))

        # ---------------- SBUF tiles ----------------
        # per-quarter tiles so a consumer's dependency covers only the DMA
        # that actually feeds it (tile-granular dep tracking)
        qx_s = [sb.tile([128, 4, 512], FP8, name=f"qx{h}", tag=f"qx{h}")
                for h in range(2)]
        kvx_s = [sb.tile([128, 4, 512], FP8, name=f"kvx{q}", tag=f"kvx{q}")
                 for q in range(4)]
        wq_s = sb.tile([128, 4, 256], FP8, name="wq", tag="wq")
        wk_s = sb.tile([128, 4, 256], FP8, name="wk", tag="wk")
        wv_s = sb.tile([128, 4, 256], FP8, name="wv", tag="wv")
        wo_s = sb.tile([128, 2, 512], BF16, name="wo", tag="wo")
        bq_s = sb.tile([128, 2], F32, name="bq", tag="bq")
        bk_s = sb.tile([128, 2], F32, name="bk", tag="bk")
        qt_s = [sb.tile([128, 1024], BF16, name=f"qt{m}", tag=f"qt{m}") for m in range(2)]
        kt_s = [sb.tile([128, 2048], BF16, name=f"kt{m}", tag=f"kt{m}") for m in range(2)]
        # v in fp8, [kv-pair, kv-parity, 4 heads x (64 dims + ones col)],
        # padded to 272 so the DoubleRow parity stride is 16B-aligned
        v_s = sb.tile([128, 8, 2, 272], FP8, name="v", tag="v")
        ot_s = [sb.tile([128, 1024], BF16, name=f"ot{m}", tag=f"ot{m}") for m in range(2)]
        # broadcast pattern for the tail norm: bpat[0, 0:64] = 1,
        # bpat[32, 64:128] = 1; rows 1-31 zero. den33 rows 1-31 stay 1.0
        # (recip-safe). Partition writes must be 0/32-aligned (verifier).
        bpat = sb.tile([33, 128], BF16, name="bpat", tag="bpat")
        den33 = sb.tile([33, 512], BF16, name="den33", tag="den33")
        l33 = sb.tile([33, 512], BF16, name="l33", tag="l33")
        recip33b = sb.tile([33, 512], BF16, name="recip33b", tag="recip33b")

        nc.vector.memset(bpat[:], 0.0)
        nc.vector.memset(bpat[0:1, 0:64], 1.0)
        nc.vector.memset(bpat[32:33, 64:128], 1.0)
        nc.vector.memset(den33[:], 1.0)
        for j in range(4):
            nc.vector.memset(v_s[:, :, :, 65 * j + 64], 1.0)
        # prime the ACT table set that holds BOTH exp and ln, so the norm
        # Ln calls never trigger a mid-stream table switch
        nc.scalar.activation(out=l33[0:1, 0:1], in_=den33[0:1, 0:1], func=AF.Ln)

        # ------------- DMAs: consumption order, spread across sequencers ----
        nc.sync.dma_start(out=wk_s[:], in_=wk_d[:])
        nc.sync.dma_start(out=kvx_s[0][:], in_=kvx_d[:, 0])
        nc.sync.dma_start(out=bk_s[:], in_=bk_d[:])
        nc.scalar.dma_start(out=wq_s[:], in_=wq_d[:])
        nc.scalar.dma_start(out=qx_s[0][:], in_=qx_d[:, 0])
        nc.scalar.dma_start(out=bq_s[:], in_=bq_d[:])
        nc.sync.dma_start(out=kvx_s[1][:], in_=kvx_d[:, 1])
        nc.scalar.dma_start(out=qx_s[1][:], in_=qx_d[:, 1])
        nc.sync.dma_start(out=kvx_s[2][:], in_=kvx_d[:, 2])
        nc.scalar.dma_start(out=wv_s[:], in_=wv_d[:])
        nc.sync.dma_start(out=kvx_s[3][:], in_=kvx_d[:, 3])
        nc.scalar.dma_start(out=wo_s[:], in_=wo_d[:])

        # ---------------- building blocks ----------------
        def qproj_group(m, t, bias_engine="vector"):
            ps = o_ps.tile([128, 512], F32, name="o", tag="o", bufs=4)
            for kk in (0, 2):
                nc.tensor.matmul(
                    ps,
                    lhsT=wq_s[:, kk:kk + 2, m * 128:(m + 1) * 128],
                    rhs=qx_s[t][:, kk:kk + 2, :],
                    start=(kk == 0), stop=(kk == 2), perf_mode=DR,
                )
            if bias_engine == "scalar":
                # ScalarE is idle during startup; bias-add there so the
                # first scores don't queue behind DVE
                nc.scalar.activation(
                    out=qt_s[m][:, t * 512:(t + 1) * 512], in_=ps,
                    func=AF.Identity, bias=bq_s[:, m:m + 1],
                )
            else:
                nc.vector.tensor_scalar_add(
                    out=qt_s[m][:, t * 512:(t + 1) * 512], in0=ps,
                    scalar1=bq_s[:, m:m + 1],
                )

        def kproj_group(m, t):
            ps = o_ps.tile([128, 512], F32, name="o", tag="o", bufs=4)
            for kk in (0, 2):
                nc.tensor.matmul(
                    ps,
                    lhsT=wk_s[:, kk:kk + 2, m * 128:(m + 1) * 128],
                    rhs=kvx_s[t][:, kk:kk + 2, :],
                    start=(kk == 0), stop=(kk == 2), perf_mode=DR,
                )
            nc.vector.tensor_scalar_add(
                out=kt_s[m][:, t * 512:(t + 1) * 512], in0=ps,
                scalar1=bk_s[:, m:m + 1],
            )

        def vproj_tile(tt):
            ps = sc_ps.tile([128, 256], F32, name="sc", tag="sc")
            c0 = (tt % 4) * 128
            for kk in (0, 2):
                nc.tensor.matmul(
                    ps,
                    lhsT=kvx_s[tt // 4][:, kk:kk + 2, c0:c0 + 128],
                    rhs=wv_s[:, kk:kk + 2, :],
                    start=(kk == 0), stop=(kk == 2), perf_mode=DR,
                )
            # fp8 cast into the interleaved layout, skipping ones columns
            pair, par = tt // 2, tt % 2
            vsl = v_s[:, pair, par, :]
            v_dst = bass.AP(tensor=vsl.tensor, offset=vsl.offset,
                            ap=[vsl.ap[0], [65, 4], [1, 64]])
            p_src = bass.AP(tensor=ps.tensor, offset=ps.offset,
                            ap=[ps.ap[0], [64, 4], [1, 64]])
            nc.vector.tensor_copy(out=v_dst, in_=p_src)

        o_tiles = {}
        e_pairs = {}
        sd_tiles = {}

        def attn_pair(m, t, j):
            oA, oB = o_tiles[(m, t)]
            e2 = e_pairs.pop((m, t, j))
            jA, jB = 2 * m, 2 * m + 1
            nc.tensor.matmul(
                oA, lhsT=v_s[:, j, :, 65 * jA:65 * jA + 65],
                rhs=e2[:, 0, :, :],
                start=(j == 0), stop=(j == 7), perf_mode=DR,
            )
            nc.tensor.matmul(
                oB, lhsT=v_s[:, j, :, 65 * jB:65 * jB + 65],
                rhs=e2[:, 1, :, :],
                start=(j == 0), stop=(j == 7), perf_mode=DR,
            )

        def _recip_act():
            # 1/den = exp(-ln(den)) on ScalarE — the exp stream has natural
            # gaps at unit boundaries, and this keeps DVE free
            nc.scalar.activation(out=l33, in_=den33, func=AF.Ln)
            nc.scalar.activation(out=recip33b, in_=l33, func=AF.Exp, scale=-1.0)

        def normA(m, t):
            oA, oB = o_tiles[(m, t)]
            nc.vector.tensor_copy(out=den33[0:1, :], in_=oA[64:65, :])
            nc.vector.tensor_copy(out=den33[32:33, :], in_=oB[64:65, :])
            _recip_act()
            sd = dpool.tile([2, 512], BF16, name="sd", tag="sd")
            nc.sync.dma_start(out=sd[0:1, :], in_=recip33b[0:1, :])
            nc.sync.dma_start(out=sd[1:2, :], in_=recip33b[32:33, :])
            sd_tiles[(m, t)] = sd

        def normB(m, t):
            qsl = slice(t * 512, (t + 1) * 512)
            oA, oB = o_tiles.pop((m, t))
            sd = sd_tiles.pop((m, t))
            bcs = small.tile([128, 512], BF16, name="bcs", tag="bcs")
            for row, base in ((0, 0), (1, 64)):
                row_ap = sd[row:row + 1, :]
                bsrc = bass.AP(tensor=row_ap.tensor, offset=row_ap.offset,
                               ap=[[0, 64], [1, 512]])
                nc.sync.dma_start(out=bcs[base:base + 64, :], in_=bsrc)
            nc.vector.tensor_mul(ot_s[m][0:64, qsl], oA[0:64, :], bcs[0:64, :])
            nc.vector.tensor_mul(ot_s[m][64:128, qsl], oB[0:64, :], bcs[64:128, :])

        def norm_tail(m, t):
            qsl = slice(t * 512, (t + 1) * 512)
            oA, oB = o_tiles.pop((m, t))
            nc.scalar.activation(out=den33[0:1, :], in_=oA[64:65, :], func=AF.Copy)
            nc.vector.tensor_copy(out=den33[32:33, :], in_=oB[64:65, :])
            _recip_act()
            # bc borrows the sc pool: it's idle once the exp stream is done
            bc = sc_ps.tile([128, 512], F32, name="sc", tag="sc")
            nc.tensor.matmul(bc, lhsT=bpat[:], rhs=recip33b[:],
                             start=True, stop=True)
            bc_s = small.tile([128, 512], F32, name="bc_s", tag="bc_s")
            nc.vector.tensor_copy(out=bc_s, in_=bc)
            nc.vector.tensor_mul(ot_s[m][0:64, qsl], oA[0:64, :], bc_s[0:64, :])
            nc.vector.tensor_mul(ot_s[m][64:128, qsl], oB[0:64, :], bc_s[64:128, :])

        fo_tiles = {}
        held_ps = {}

        def _outproj_mm(ps, t2, mo, m, start, stop):
            nc.tensor.matmul(
                ps,
                lhsT=wo_s[:, m, mo * 128:(mo + 1) * 128],
                rhs=ot_s[m][:, t2 * 512:(t2 + 1) * 512],
                start=start, stop=stop,
            )

        def _outproj_emit(ps, t2, mo, engine, dma=None):
            if t2 not in fo_tiles:
                fo_tiles[t2] = small.tile([128, 4, 512], F32, name="fo",
                                          tag="fo", bufs=2)
            fo = fo_tiles[t2]
            if engine == "vector":
                nc.vector.tensor_copy(out=fo[:, mo, :], in_=ps)
            else:
                nc.scalar.activation(out=fo[:, mo, :], in_=ps, func=AF.Copy)
            if dma == "split":
                # last chunk: halve across both sequencers to shorten the tail
                nc.sync.dma_start(
                    out=out_d[mo * 128:mo * 128 + 64, t2 * 512:(t2 + 1) * 512],
                    in_=fo[0:64, mo, :],
                )
                nc.scalar.dma_start(
                    out=out_d[mo * 128 + 64:(mo + 1) * 128, t2 * 512:(t2 + 1) * 512],
                    in_=fo[64:128, mo, :],
                )
            else:
                dma_eng = nc.scalar if dma == "scalar" else nc.sync
                dma_eng.dma_start(
                    out=out_d[mo * 128:(mo + 1) * 128, t2 * 512:(t2 + 1) * 512],
                    in_=fo[:, mo, :],
                )

        def outproj_group(t2, mo, engine="vector", dma=None):
            ps = o_ps.tile([128, 512], F32, name="o", tag="o", bufs=4)
            _outproj_mm(ps, t2, mo, 0, True, False)
            _outproj_mm(ps, t2, mo, 1, False, True)
            _outproj_emit(ps, t2, mo, engine, dma)

        def outproj_first(t2, mo):
            ps = o_ps.tile([128, 512], F32, name="o", tag="o", bufs=4)
            _outproj_mm(ps, t2, mo, 0, True, False)
            held_ps[(t2, mo)] = ps

        def outproj_finish(t2, mo, engine, dma=None):
            ps = held_ps.pop((t2, mo))
            _outproj_mm(ps, t2, mo, 1, False, True)
            _outproj_emit(ps, t2, mo, engine, dma)

        # ---------------- pipelined schedule ----------------
        units = [(0, 0), (1, 0), (0, 1), (1, 1)]
        iters = [(u, i) for u in units for i in range(16)]

        extra = {g: [] for g in range(64)}
        # attn@V pair j of unit u fires at g = 16u + 2j + 3: its exp wait is
        # already satisfied, so it never head-of-line blocks the PE queue.
        tail_attn = []
        for u, (m_, t_) in enumerate(units):
            for j in range(8):
                g = 16 * u + 2 * j + 3
                fn = (lambda m_=m_, t_=t_, j=j: attn_pair(m_, t_, j))
                if g < 64:
                    extra[g].append(fn)
                else:
                    tail_attn.append(fn)
        # vproj(tt) must be emitted before the attn pair that reads it
        # (attn pair j reads tiles 2j, 2j+1 at g=2j+3 — program order is the
        # dependency order)
        # front-load vproj: tiles 0-1 fill the early-pipeline PE idle, the
        # rest run one per iteration (all well before their attn pair reads)
        extra[0].append(lambda: vproj_tile(0))
        extra[0].append(lambda: vproj_tile(1))
        for tt in range(2, 16):
            extra[tt - 1].append(lambda tt=tt: vproj_tile(tt))
        extra[1].append(lambda: kproj_group(0, 1))
        extra[5].append(lambda: kproj_group(0, 2))
        extra[9].append(lambda: kproj_group(0, 3))
        extra[11].append(lambda: qproj_group(1, 0))
        extra[13].append(lambda: kproj_group(1, 0))
        extra[16].append(lambda: kproj_group(1, 1))
        extra[17].append(lambda: kproj_group(1, 2))
        extra[17].append(lambda: kproj_group(1, 3))
        extra[18].append(lambda: normA(0, 0))
        extra[21].append(lambda: normB(0, 0))
        extra[26].append(lambda: qproj_group(0, 1))
        extra[34].append(lambda: normA(1, 0))
        extra[37].append(lambda: normB(1, 0))
        extra[38].append(lambda: qproj_group(1, 1))
        extra[41].append(lambda: outproj_group(0, 0))
        extra[43].append(lambda: outproj_group(0, 1))
        extra[45].append(lambda: outproj_group(0, 2))
        extra[47].append(lambda: outproj_group(0, 3))
        extra[50].append(lambda: normA(0, 1))
        extra[53].append(lambda: normB(0, 1))
        extra[57].append(lambda: outproj_first(1, 0))
        extra[59].append(lambda: outproj_first(1, 1))

        # kvx quarter 0 lands before qx half 0 — emit kproj first so the PE
        # starts as soon as data arrives
        kproj_group(0, 0)
        qproj_group(0, 0, bias_engine="scalar")

        sc_tiles = {}

        def emit_scores(g):
            (m, t), i = iters[g]
            ksl = slice(i * 128, (i + 1) * 128)
            qsl = slice(t * 512, (t + 1) * 512)
            sc = sc_ps.tile([128, 2, 512], F32, name="sc", tag="sc")
            nc.tensor.matmul(
                sc[:, 0, :], lhsT=kt_s[m][0:64, ksl], rhs=qt_s[m][0:64, qsl],
                start=True, stop=True, tile_position=(0, 0),
            )
            nc.tensor.matmul(
                sc[:, 1, :], lhsT=kt_s[m][64:128, ksl], rhs=qt_s[m][64:128, qsl],
                start=True, stop=True, tile_position=(64, 0),
            )
            sc_tiles[g] = sc

        emit_scores(0)
        e_cur = None
        for g in range(64):
            (m, t), i = iters[g]
            if g + 1 < 64:
                emit_scores(g + 1)
            sc = sc_tiles.pop(g)
            if i % 2 == 0:
                e_cur = epool.tile([128, 2, 2, 512], FP8, name="e", tag="e")
                e_pairs[(m, t, i // 2)] = e_cur
            nc.scalar.activation(out=e_cur[:, :, i % 2, :], in_=sc[:],
                                 func=AF.Exp, scale=0.125)
            if i == 0:
                oA = o_ps.tile([65, 512], F32, name="o", tag="o", bufs=4)
                oB = o_ps.tile([65, 512], F32, name="o", tag="o", bufs=4)
                o_tiles[(m, t)] = (oA, oB)
            for fn in extra.get(g, ()):
                fn()

        for fn in tail_attn:
            fn()
        norm_tail(1, 1)
        outproj_finish(1, 0, "scalar", dma="scalar")
        outproj_finish(1, 1, "vector")
        outproj_group(1, 2, engine="scalar", dma="scalar")
        outproj_group(1, 3, engine="vector", dma="split")

    _split_multi_waits(nc)
    return nc


_PROGRAM = None


def _get_program() -> bass.Bass:
    global _PROGRAM
    if _PROGRAM is None:
        _PROGRAM = _build_program()
    return _PROGRAM


def _prep_core_inputs(c, q, kv, Wqkv, bqkv, Wout):
    b, g = c // 2, c % 2
    cs = slice(256 * g, 256 * g + 256)
    ks = slice(512 + 256 * g, 512 + 256 * g + 256)
    vs = slice(1024 + 256 * g, 1024 + 256 * g + 256)

    def chunk_w(wT):  # [512, 256] -> [128, 4, 256], row 128k+p -> [p, k]
        return np.ascontiguousarray(wT.reshape(4, 128, -1).transpose(1, 0, 2))

    qxp = q[b].reshape(4, 128, 2, 512).transpose(1, 2, 0, 3)
    kvxp = kv[b].reshape(4, 128, 4, 512).transpose(1, 2, 0, 3)
    return {
        "qx": np.ascontiguousarray(qxp).astype(NP_FP8),
        "kvx": np.ascontiguousarray(kvxp).astype(NP_FP8),
        "wq": chunk_w(Wqkv[cs, :].T).astype(NP_FP8),
        "wk": chunk_w(Wqkv[ks, :].T).astype(NP_FP8),
        "wv": chunk_w(Wqkv[vs, :].T).astype(NP_FP8),
        "wo": np.ascontiguousarray(
            Wout[:, cs].T.reshape(2, 128, 512).transpose(1, 0, 2)).astype(NP_BF16),
        "bq": np.ascontiguousarray(bqkv[cs].reshape(2, 128).T).astype(np.float32),
        "bk": np.ascontiguousarray(bqkv[ks].reshape(2, 128).T).astype(np.float32),
    }


def kernel(q, kv, Wqkv, bqkv, Wout, bout):
    q = np.asarray(q, np.float32)
    kv = np.asarray(kv, np.float32)
    Wqkv = np.asarray(Wqkv, np.float32)
    bqkv = np.asarray(bqkv, np.float32)
    Wout = np.asarray(Wout, np.float32)
    bout = np.asarray(bout, np.float32)

    nc = _get_program()
    in_maps = [_prep_core_inputs(c, q, kv, Wqkv, bqkv, Wout) for c in range(8)]
    res = run_bass_kernel_spmd(nc, in_maps, list(range(8))).results

    # V-bias folds through softmax (rows sum to 1): bout' = bout + Wout @ bv
    bout_adj = bout + Wout @ bqkv[1024:1536]
    out = np.empty((4, 512, 32, 32), np.float32)
    for b in range(4):
        o = res[2 * b]["out"] + res[2 * b + 1]["out"] + bout_adj[:, None]
        out[b] = o.reshape(512, 32, 32)
    return out


# revision 49
# speedup vs baseline: 1.0625x; 1.0394x over previous
"""Bass/Trainium2 kernel for BiDirectionalCrossAttention (8-core SPMD).

Sharding: 8 cores = 4 batches x 2 head-groups (4 heads each).
Per core (batch b, head-group g of 4 heads):
  - Q/K/V projections as fp8e4m3 DoubleRow matmuls (K=256 per matmul:
    channel-chunk pairs), biases added in f32, qt/kt stored bf16
  - V stored fp8 in [token, chan] layout with memset ones-columns
    interleaved per head (softmax denominator rides the attn@V matmul)
  - scoresT[kv, q] per head in bf16 (row-tiled concurrent pairs), exp on
    ScalarE writing fp8e4m3 directly
  - attn@V as fp8 DoubleRow matmuls (K=256: two kv tiles per matmul),
    emitted 2 iterations late so their waits are pre-satisfied (no
    head-of-line blocking on the PE queue)
  - normalization: denominator rows -> reciprocal -> bf16; mid-stream
    units broadcast 1/den via a DRAM bounce (0-stride partition read),
    the final unit via a PE broadcast matmul (latency-critical tail)
  - partial output projection Wout[:, cols_g] @ out_g -> [512, 1024]
Host sums the two partials per batch and adds the folded bias
bout' = bout + Wout @ bv (V-bias commutes through softmax).
"""

import sys
import os

for _p in ("/opt/trn_rl_repo", "/root/.axon_site/_ro/trn_rl_repo"):
    if os.path.isdir(_p) and _p not in sys.path:
        sys.path.append(_p)

import numpy as np
import ml_dtypes

import concourse.bass as bass
import concourse.mybir as mybir
import concourse.tile as tile
from concourse.bass_utils import run_bass_kernel_spmd

BF16 = mybir.dt.bfloat16
F32 = mybir.dt.float32
FP8 = mybir.dt.float8e4
NP_BF16 = ml_dtypes.bfloat16
NP_FP8 = ml_dtypes.float8_e4m3

AF = mybir.ActivationFunctionType
DR = mybir.MatmulPerfMode.DoubleRow


def _split_multi_waits(nc: bass.Bass) -> None:
    """The walrus build here allows only one sync-wait per instruction.
    Tile attaches several; hoist the extras onto same-engine NOPs placed
    immediately before the instruction (same per-engine program order)."""
    uid = 0
    for f in nc.m.functions:
        for bb in f.blocks:
            insts = bb.instructions
            out = []
            changed = False
            for inst in insts:
                si = inst.sync_info
                if si is not None and si.on_wait is not None and len(si.on_wait) > 1:
                    waits = list(si.on_wait)
                    for w in waits[:-1]:
                        nop = mybir.InstNoOp(
                            name=f"splitwait-{uid}",
                            engine=inst.engine,
                            ins=[],
                            outs=[],
                            sync_info=mybir.SyncInfo(on_wait=[w], on_update=[]),
                        )
                        uid += 1
                        out.append(nop)
                    inst.sync_info = mybir.SyncInfo(
                        on_wait=[waits[-1]], on_update=list(si.on_update or [])
                    )
                    changed = True
                out.append(inst)
            if changed:
                bb.instructions = out


def _build_program() -> bass.Bass:
    nc = bass.Bass()

    # host-prepped, partition-contiguous layouts
    qx_d = nc.declare_dram_parameter("qx", [128, 2, 4, 512], FP8, isOutput=False)
    kvx_d = nc.declare_dram_parameter("kvx", [128, 4, 4, 512], FP8, isOutput=False)
    wq_d = nc.declare_dram_parameter("wq", [128, 4, 256], FP8, isOutput=False)
    wk_d = nc.declare_dram_parameter("wk", [128, 4, 256], FP8, isOutput=False)
    wv_d = nc.declare_dram_parameter("wv", [128, 4, 256], FP8, isOutput=False)
    wo_d = nc.declare_dram_parameter("wo", [128, 2, 512], BF16, isOutput=False)
    bq_d = nc.declare_dram_parameter("bq", [128, 2], F32, isOutput=False)
    bk_d = nc.declare_dram_parameter("bk", [128, 2], F32, isOutput=False)
    out_d = nc.declare_dram_parameter("out", [512, 1024], F32, isOutput=True)

    from contextlib import ExitStack

    with tile.TileContext(nc) as tc, ExitStack() as ctx:
        sb = ctx.enter_context(tc.tile_pool(name="sb", bufs=1))
        epool = ctx.enter_context(tc.tile_pool(name="epool", bufs=3))
        small = ctx.enter_context(tc.tile_pool(name="small", bufs=4))
        dpool = ctx.enter_context(tc.tile_pool(name="dram", bufs=2, space="DRAM"))
        # PSUM budget (8 banks): "sc" 2 slots x [128,2,512] (2 banks) = 4,
        # "o" 4 slots x 1 bank = 4 (2 units' accumulators overlap at
        # boundaries; proj psums churn through free slots mid-unit).
        sc_ps = ctx.enter_context(tc.tile_pool(name="scps", bufs=2, space="PSUM"))
        o_ps = ctx.enter_context(tc.tile_pool(name="ops", bufs=4, space="PSUM"))

        # ---------------- SBUF tiles ----------------
        # per-quarter tiles so a consumer's dependency covers only the DMA
        # that actually feeds it (tile-granular dep tracking)
        qx_s = [sb.tile([128, 4, 512], FP8, name=f"qx{h}", tag=f"qx{h}")
                for h in range(2)]
        kvx_s = [sb.tile([128, 4, 512], FP8, name=f"kvx{q}", tag=f"kvx{q}")
                 for q in range(4)]
        wq_s = sb.tile([128, 4, 256], FP8, name="wq", tag="wq")
        wk_s = sb.tile([128, 4, 256], FP8, name="wk", tag="wk")
        wv_s = sb.tile([128, 4, 256], FP8, name="wv", tag="wv")
        wo_s = sb.tile([128, 2, 512], BF16, name="wo", tag="wo")
        bq_s = sb.tile([128, 2], F32, name="bq", tag="bq")
        bk_s = sb.tile([128, 2], F32, name="bk", tag="bk")
        qt_s = [sb.tile([128, 1024], BF16, name=f"qt{m}", tag=f"qt{m}") for m in range(2)]
        kt_s = [sb.tile([128, 2048], BF16, name=f"kt{m}", tag=f"kt{m}") for m in range(2)]
        # v in fp8, [kv-pair, kv-parity, 4 heads x (64 dims + ones col)],
        # padded to 272 so the DoubleRow parity stride is 16B-aligned
        v_s = sb.tile([128, 8, 2, 272], FP8, name="v", tag="v")
        ot_s = [sb.tile([128, 1024], BF16, name=f"ot{m}", tag=f"ot{m}") for m in range(2)]
        # broadcast pattern for the tail norm: bpat[0, 0:64] = 1,
        # bpat[32, 64:128] = 1; rows 1-31 zero. den33 rows 1-31 stay 1.0
        # (recip-safe). Partition writes must be 0/32-aligned (verifier).
        bpat = sb.tile([33, 128], BF16, name="bpat", tag="bpat")
        den33 = sb.tile([33, 512], BF16, name="den33", tag="den33")
        l33 = sb.tile([33, 512], BF16, name="l33", tag="l33")
        recip33b = sb.tile([33, 512], BF16, name="recip33b", tag="recip33b")

        nc.vector.memset(bpat[:], 0.0)
        nc.vector.memset(bpat[0:1, 0:64], 1.0)
        nc.vector.memset(bpat[32:33, 64:128], 1.0)
        nc.vector.memset(den33[:], 1.0)
        for j in range(4):
            nc.vector.memset(v_s[:, :, :, 65 * j + 64], 1.0)
        # prime the ACT table set that holds BOTH exp and ln, so the norm
        # Ln calls never trigger a mid-stream table switch
        nc.scalar.activation(out=l33[0:1, 0:1], in_=den33[0:1, 0:1], func=AF.Ln)

        # ------------- DMAs: consumption order, spread across sequencers ----
        nc.sync.dma_start(out=wk_s[:], in_=wk_d[:])
        nc.sync.dma_start(out=kvx_s[0][:], in_=kvx_d[:, 0])
        nc.sync.dma_start(out=bk_s[:], in_=bk_d[:])
        nc.scalar.dma_start(out=wq_s[:], in_=wq_d[:])
        nc.scalar.dma_start(out=qx_s[0][:], in_=qx_d[:, 0])
        nc.scalar.dma_start(out=bq_s[:], in_=bq_d[:])
        nc.sync.dma_start(out=kvx_s[1][:], in_=kvx_d[:, 1])
        nc.scalar.dma_start(out=qx_s[1][:], in_=qx_d[:, 1])
        nc.sync.dma_start(out=kvx_s[2][:], in_=kvx_d[:, 2])
        nc.scalar.dma_start(out=wv_s[:], in_=wv_d[:])
        nc.sync.dma_start(out=kvx_s[3][:], in_=kvx_d[:, 3])
        nc.scalar.dma_start(out=wo_s[:], in_=wo_d[:])

        # ---------------- building blocks ----------------
        def qproj_group(m, t, bias_engine="vector"):
            ps = o_ps.tile([128, 512], F32, name="o", tag="o", bufs=4)
            for kk in (0, 2):
                nc.tensor.matmul(
                    ps,
                    lhsT=wq_s[:, kk:kk + 2, m * 128:(m + 1) * 128],
                    rhs=qx_s[t][:, kk:kk + 2, :],
                    start=(kk == 0), stop=(kk == 2), perf_mode=DR,
                )
            if bias_engine == "scalar":
                # ScalarE is idle during startup; bias-add there so the
                # first scores don't queue behind DVE
                nc.scalar.activation(
                    out=qt_s[m][:, t * 512:(t + 1) * 512], in_=ps,
                    func=AF.Identity, bias=bq_s[:, m:m + 1],
                )
            else:
                nc.vector.tensor_scalar_add(
                    out=qt_s[m][:, t * 512:(t + 1) * 512], in0=ps,
                    scalar1=bq_s[:, m:m + 1],
                )

        def kproj_group(m, t):
            ps = o_ps.tile([128, 512], F32, name="o", tag="o", bufs=4)
            for kk in (0, 2):
                nc.tensor.matmul(
                    ps,
                    lhsT=wk_s[:, kk:kk + 2, m * 128:(m + 1) * 128],
                    rhs=kvx_s[t][:, kk:kk + 2, :],
                    start=(kk == 0), stop=(kk == 2), perf_mode=DR,
                )
            nc.vector.tensor_scalar_add(
                out=kt_s[m][:, t * 512:(t + 1) * 512], in0=ps,
                scalar1=bk_s[:, m:m + 1],
            )

        def vproj_tile(tt):
            ps = sc_ps.tile([128, 256], F32, name="sc", tag="sc")
            c0 = (tt % 4) * 128
            for kk in (0, 2):
                nc.tensor.matmul(
                    ps,
                    lhsT=kvx_s[tt // 4][:, kk:kk + 2, c0:c0 + 128],
                    rhs=wv_s[:, kk:kk + 2, :],
                    start=(kk == 0), stop=(kk == 2), perf_mode=DR,
                )
            # fp8 cast into the interleaved layout, skipping ones columns
            pair, par = tt // 2, tt % 2
            vsl = v_s[:, pair, par, :]
            v_dst = bass.AP(tensor=vsl.tensor, offset=vsl.offset,
                            ap=[vsl.ap[0], [65, 4], [1, 64]])
            p_src = bass.AP(tensor=ps.tensor, offset=ps.offset,
                            ap=[ps.ap[0], [64, 4], [1, 64]])
            nc.vector.tensor_copy(out=v_dst, in_=p_src)

        o_tiles = {}
        e_pairs = {}
        sd_tiles = {}

        def attn_pair(m, t, j):
            oA, oB = o_tiles[(m, t)]
            e2 = e_pairs.pop((m, t, j))
            jA, jB = 2 * m, 2 * m + 1
            nc.tensor.matmul(
                oA, lhsT=v_s[:, j, :, 65 * jA:65 * jA + 65],
                rhs=e2[:, 0, :, :],
                start=(j == 0), stop=(j == 7), perf_mode=DR,
            )
            nc.tensor.matmul(
                oB, lhsT=v_s[:, j, :, 65 * jB:65 * jB + 65],
                rhs=e2[:, 1, :, :],
                start=(j == 0), stop=(j == 7), perf_mode=DR,
            )

        def _recip_act():
            # 1/den = exp(-ln(den)) on ScalarE — the exp stream has natural
            # gaps at unit boundaries, and this keeps DVE free
            nc.scalar.activation(out=l33, in_=den33, func=AF.Ln)
            nc.scalar.activation(out=recip33b, in_=l33, func=AF.Exp, scale=-1.0)

        def normA(m, t):
            oA, oB = o_tiles[(m, t)]
            nc.vector.tensor_copy(out=den33[0:1, :], in_=oA[64:65, :])
            nc.vector.tensor_copy(out=den33[32:33, :], in_=oB[64:65, :])
            _recip_act()
            sd = dpool.tile([2, 512], BF16, name="sd", tag="sd")
            nc.sync.dma_start(out=sd[0:1, :], in_=recip33b[0:1, :])
            nc.sync.dma_start(out=sd[1:2, :], in_=recip33b[32:33, :])
            sd_tiles[(m, t)] = sd

        def normB(m, t):
            qsl = slice(t * 512, (t + 1) * 512)
            oA, oB = o_tiles.pop((m, t))
            sd = sd_tiles.pop((m, t))
            bcs = small.tile([128, 512], BF16, name="bcs", tag="bcs")
            for row, base in ((0, 0), (1, 64)):
                row_ap = sd[row:row + 1, :]
                bsrc = bass.AP(tensor=row_ap.tensor, offset=row_ap.offset,
                               ap=[[0, 64], [1, 512]])
                nc.sync.dma_start(out=bcs[base:base + 64, :], in_=bsrc)
            nc.vector.tensor_mul(ot_s[m][0:64, qsl], oA[0:64, :], bcs[0:64, :])
            nc.vector.tensor_mul(ot_s[m][64:128, qsl], oB[0:64, :], bcs[64:128, :])

        def norm_tail(m, t):
            qsl = slice(t * 512, (t + 1) * 512)
            oA, oB = o_tiles.pop((m, t))
            nc.scalar.activation(out=den33[0:1, :], in_=oA[64:65, :], func=AF.Copy)
            nc.vector.tensor_copy(out=den33[32:33, :], in_=oB[64:65, :])
            _recip_act()
            # bc borrows the sc pool: it's idle once the exp stream is done
            bc = sc_ps.tile([128, 512], F32, name="sc", tag="sc")
            nc.tensor.matmul(bc, lhsT=bpat[:], rhs=recip33b[:],
                             start=True, stop=True)
            bc_s = small.tile([128, 512], F32, name="bc_s", tag="bc_s")
            nc.vector.tensor_copy(out=bc_s, in_=bc)
            nc.vector.tensor_mul(ot_s[m][0:64, qsl], oA[0:64, :], bc_s[0:64, :])
            nc.vector.tensor_mul(ot_s[m][64:128, qsl], oB[0:64, :], bc_s[64:128, :])

        fo_tiles = {}
        held_ps = {}

        def _outproj_mm(ps, t2, mo, m, start, stop):
            nc.tensor.matmul(
                ps,
                lhsT=wo_s[:, m, mo * 128:(mo + 1) * 128],
                rhs=ot_s[m][:, t2 * 512:(t2 + 1) * 512],
                start=start, stop=stop,
            )

        def _outproj_emit(ps, t2, mo, engine, dma=None):
            if t2 not in fo_tiles:
                fo_tiles[t2] = small.tile([128, 4, 512], F32, name="fo",
                                          tag="fo", bufs=2)
            fo = fo_tiles[t2]
            if engine == "vector":
                nc.vector.tensor_copy(out=fo[:, mo, :], in_=ps)
            else:
                nc.scalar.activation(out=fo[:, mo, :], in_=ps, func=AF.Copy)
            if dma == "split":
                # last chunk: halve across both sequencers to shorten the tail
                nc.sync.dma_start(
                    out=out_d[mo * 128:mo * 128 + 64, t2 * 512:(t2 + 1) * 512],
                    in_=fo[0:64, mo, :],
                )
                nc.scalar.dma_start(
                    out=out_d[mo * 128 + 64:(mo + 1) * 128, t2 * 512:(t2 + 1) * 512],
                    in_=fo[64:128, mo, :],
                )
            else:
                dma_eng = nc.scalar if dma == "scalar" else nc.sync
                dma_eng.dma_start(
                    out=out_d[mo * 128:(mo + 1) * 128, t2 * 512:(t2 + 1) * 512],
                    in_=fo[:, mo, :],
                )

        def outproj_group(t2, mo, engine="vector", dma=None):
            ps = o_ps.tile([128, 512], F32, name="o", tag="o", bufs=4)
            _outproj_mm(ps, t2, mo, 0, True, False)
            _outproj_mm(ps, t2, mo, 1, False, True)
            _outproj_emit(ps, t2, mo, engine, dma)

        def outproj_first(t2, mo):
            ps = o_ps.tile([128, 512], F32, name="o", tag="o", bufs=4)
            _outproj_mm(ps, t2, mo, 0, True, False)
            held_ps[(t2, mo)] = ps

        def outproj_finish(t2, mo, engine, dma=None):
            ps = held_ps.pop((t2, mo))
            _outproj_mm(ps, t2, mo, 1, False, True)
            _outproj_emit(ps, t2, mo, engine, dma)

        # ---------------- pipelined schedule ----------------
        units = [(0, 0), (1, 0), (0, 1), (1, 1)]
        iters = [(u, i) for u in units for i in range(16)]

        extra = {g: [] for g in range(64)}
        # attn@V pair j of unit u fires at g = 16u + 2j + 3: its exp wait is
        # already satisfied, so it never head-of-line blocks the PE queue.
        tail_attn = []
        for u, (m_, t_) in enumerate(units):
            for j in range(8):
                g = 16 * u + 2 * j + 3
                fn = (lambda m_=m_, t_=t_, j=j: attn_pair(m_, t_, j))
                if g < 64:
                    extra[g].append(fn)
                else:
                    tail_attn.append(fn)
        # vproj(tt) must be emitted before the attn pair that reads it
        # (attn pair j reads tiles 2j, 2j+1 at g=2j+3 — program order is the
        # dependency order)
        # front-load vproj: tiles 0-1 fill the early-pipeline PE idle, the
        # rest run one per iteration (all well before their attn pair reads)
        extra[0].append(lambda: vproj_tile(0))
        extra[0].append(lambda: vproj_tile(1))
        for tt in range(2, 16):
            extra[tt - 1].append(lambda tt=tt: vproj_tile(tt))
        extra[1].append(lambda: kproj_group(0, 1))
        extra[5].append(lambda: kproj_group(0, 2))
        extra[9].append(lambda: kproj_group(0, 3))
        extra[11].append(lambda: qproj_group(1, 0))
        extra[13].append(lambda: kproj_group(1, 0))
        extra[16].append(lambda: kproj_group(1, 1))
        extra[17].append(lambda: kproj_group(1, 2))
        extra[17].append(lambda: kproj_group(1, 3))
        extra[18].append(lambda: normA(0, 0))
        extra[21].append(lambda: normB(0, 0))
        extra[26].append(lambda: qproj_group(0, 1))
        extra[34].append(lambda: normA(1, 0))
        extra[37].append(lambda: normB(1, 0))
        extra[38].append(lambda: qproj_group(1, 1))
        extra[41].append(lambda: outproj_group(0, 0))
        extra[43].append(lambda: outproj_group(0, 1))
        extra[45].append(lambda: outproj_group(0, 2))
        extra[47].append(lambda: outproj_group(0, 3))
        extra[50].append(lambda: normA(0, 1))
        extra[53].append(lambda: normB(0, 1))
        extra[58].append(lambda: outproj_first(1, 0))
        extra[60].append(lambda: outproj_first(1, 1))

        # kvx quarter 0 lands before qx half 0 — emit kproj first so the PE
        # starts as soon as data arrives
        kproj_group(0, 0)
        qproj_group(0, 0, bias_engine="scalar")

        sc_tiles = {}

        def emit_scores(g):
            (m, t), i = iters[g]
            ksl = slice(i * 128, (i + 1) * 128)
            qsl = slice(t * 512, (t + 1) * 512)
            sc = sc_ps.tile([128, 2, 512], F32, name="sc", tag="sc")
            nc.tensor.matmul(
                sc[:, 0, :], lhsT=kt_s[m][0:64, ksl], rhs=qt_s[m][0:64, qsl],
                start=True, stop=True, tile_position=(0, 0),
            )
            nc.tensor.matmul(
                sc[:, 1, :], lhsT=kt_s[m][64:128, ksl], rhs=qt_s[m][64:128, qsl],
                start=True, stop=True, tile_position=(64, 0),
            )
            sc_tiles[g] = sc

        emit_scores(0)
        e_cur = None
        for g in range(64):
            (m, t), i = iters[g]
            if g + 1 < 64:
                emit_scores(g + 1)
            sc = sc_tiles.pop(g)
            if i % 2 == 0:
                e_cur = epool.tile([128, 2, 2, 512], FP8, name="e", tag="e")
                e_pairs[(m, t, i // 2)] = e_cur
            nc.scalar.activation(out=e_cur[:, :, i % 2, :], in_=sc[:],
                                 func=AF.Exp, scale=0.125)
            if i == 0:
                oA = o_ps.tile([65, 512], F32, name="o", tag="o", bufs=4)
                oB = o_ps.tile([65, 512], F32, name="o", tag="o", bufs=4)
                o_tiles[(m, t)] = (oA, oB)
            for fn in extra.get(g, ()):
                fn()

        for fn in tail_attn:
            fn()
        norm_tail(1, 1)
        outproj_finish(1, 0, "scalar", dma="scalar")
        outproj_finish(1, 1, "vector")
        outproj_group(1, 2, engine="scalar", dma="scalar")
        outproj_group(1, 3, engine="vector", dma="split")

    _split_multi_waits(nc)
    return nc


_PROGRAM = None


def _get_program() -> bass.Bass:
    global _PROGRAM
    if _PROGRAM is None:
        _PROGRAM = _build_program()
    return _PROGRAM


def _prep_core_inputs(c, q, kv, Wqkv, bqkv, Wout):
    b, g = c // 2, c % 2
    cs = slice(256 * g, 256 * g + 256)
    ks = slice(512 + 256 * g, 512 + 256 * g + 256)
    vs = slice(1024 + 256 * g, 1024 + 256 * g + 256)

    def chunk_w(wT):  # [512, 256] -> [128, 4, 256], row 128k+p -> [p, k]
        return np.ascontiguousarray(wT.reshape(4, 128, -1).transpose(1, 0, 2))

    qxp = q[b].reshape(4, 128, 2, 512).transpose(1, 2, 0, 3)
    kvxp = kv[b].reshape(4, 128, 4, 512).transpose(1, 2, 0, 3)
    return {
        "qx": np.ascontiguousarray(qxp).astype(NP_FP8),
        "kvx": np.ascontiguousarray(kvxp).astype(NP_FP8),
        "wq": chunk_w(Wqkv[cs, :].T).astype(NP_FP8),
        "wk": chunk_w(Wqkv[ks, :].T).astype(NP_FP8),
        "wv": chunk_w(Wqkv[vs, :].T).astype(NP_FP8),
        "wo": np.ascontiguousarray(
            Wout[:, cs].T.reshape(2, 128, 512).transpose(1, 0, 2)).astype(NP_BF16),
        "bq": np.ascontiguousarray(bqkv[cs].reshape(2, 128).T).astype(np.float32),
        "bk": np.ascontiguousarray(bqkv[ks].reshape(2, 128).T).astype(np.float32),
    }


def kernel(q, kv, Wqkv, bqkv, Wout, bout):
    q = np.asarray(q, np.float32)
    kv = np.asarray(kv, np.float32)
    Wqkv = np.asarray(Wqkv, np.float32)
    bqkv = np.asarray(bqkv, np.float32)
    Wout = np.asarray(Wout, np.float32)
    bout = np.asarray(bout, np.float32)

    nc = _get_program()
    in_maps = [_prep_core_inputs(c, q, kv, Wqkv, bqkv, Wout) for c in range(8)]
    res = run_bass_kernel_spmd(nc, in_maps, list(range(8))).results

    # V-bias folds through softmax (rows sum to 1): bout' = bout + Wout @ bv
    bout_adj = bout + Wout @ bqkv[1024:1536]
    out = np.empty((4, 512, 32, 32), np.float32)
    for b in range(4):
        o = res[2 * b]["out"] + res[2 * b + 1]["out"] + bout_adj[:, None]
        out[b] = o.reshape(512, 32, 32)
    return out
